# revision 3
# baseline (speedup 1.0000x reference)
"""Dense linear attention (elu+1 feature map) Trainium2 Bass kernel.

Problem: B=8, T=4096, D=1024, H=64.
  q = phi(x @ W_q), k = phi(x @ W_k), v = x @ W_v          (phi = elu+1)
  S_t = S_{t-1} + k_t v_t^T ; z_t = z_{t-1} + k_t
  o_t = (S_t q_t) / max(z_t . q_t, 1e-6)
  y = o @ W_o

Strategy: data-parallel over batch, one batch element per NeuronCore (8 cores).
Per core, chunked linear attention with chunk C=128:
  A^T[s,t] = k_s . q_t  (masked s<=t)   -> intra-chunk attention
  O^T = V^T A_m + S_prev^T q            -> two accumulated matmuls
  denom[t] = sum_s A_m[s,t] + z_prev . q_t
  S += K^T V (with ones column for z), kept in fp32 SBUF via per-chunk dS
All matmuls in bf16 with fp32 PSUM accumulation; x transposed on PE in fp32.
"""

import numpy as np

import concourse.bass as bass
import concourse.mybir as mybir
import concourse.tile as tile
from concourse import bacc
from concourse.bass_utils import run_bass_kernel_spmd
from concourse.masks import make_identity, make_upper_triangular

F32 = mybir.dt.float32
BF16 = mybir.dt.bfloat16
AF = mybir.ActivationFunctionType

B, T, D, H = 8, 4096, 1024, 64
C = 128                 # chunk (recurrence step block)
BLK = 512               # projection block: 4 chunks
N_BLK = T // BLK        # 8
N_CH = BLK // C         # 4 chunks per block
DJ = D // 128           # 8 contraction sub-tiles


def build_nc():
    nc = bacc.Bacc("TRN2", target_bir_lowering=False, debug=False)

    x_d = nc.dram_tensor("x", [T, D], F32, kind="ExternalInput")
    wq_d = nc.dram_tensor("wq", [D, H], F32, kind="ExternalInput")
    wk_d = nc.dram_tensor("wk", [D, H], F32, kind="ExternalInput")
    wv_d = nc.dram_tensor("wv", [D, H], F32, kind="ExternalInput")
    wo_d = nc.dram_tensor("wo", [H, D], F32, kind="ExternalInput")
    y_d = nc.dram_tensor("y", [T, D], F32, kind="ExternalOutput")

    with tile.TileContext(nc) as tc:
        with (
            tc.tile_pool(name="const", bufs=1) as const,
            tc.tile_pool(name="xin", bufs=3) as xin,
            tc.tile_pool(name="xtbf", bufs=2) as xtbf,
            tc.tile_pool(name="phi", bufs=2) as phip,
            tc.tile_pool(name="chunk", bufs=3) as chp,
            tc.tile_pool(name="state", bufs=2) as stp,
            tc.tile_pool(name="yout", bufs=4) as yp,
            tc.tile_pool(name="ps_xt", bufs=2, space="PSUM") as ps_xt,
            tc.tile_pool(name="ps_qk", bufs=1, space="PSUM") as ps_qk,
            tc.tile_pool(name="ps_y", bufs=2, space="PSUM") as ps_y,
            tc.tile_pool(name="ps_sm", bufs=3, space="PSUM") as ps_sm,
        ):
            # ---- constants / weights ----
            ident_f = const.tile([128, 128], F32, tag="identf")
            make_identity(nc, ident_f[:])
            ident_b = const.tile([128, 128], BF16, tag="identb")
            make_identity(nc, ident_b[:])
            # mask[s,t] = 1.0 where s <= t (upper triangular incl diagonal)
            mask = const.tile([128, 128], F32, tag="mask")
            make_upper_triangular(nc, mask[:], val=1.0, diag=True)
            ones_b = const.tile([128, 1], BF16, tag="ones")
            nc.vector.memset(ones_b[:], 1.0)

            # weight staging (fp32) and bf16 conversion
            wq_st = const.tile([128, DJ, H], F32, tag="wqst")
            wk_st = const.tile([128, DJ, H], F32, tag="wkst")
            wv_st = const.tile([128, DJ, H], F32, tag="wvst")
            wo_st = const.tile([H, D], F32, tag="wost")
            nc.sync.dma_start(wq_st[:], wq_d.rearrange("(j p) h -> p j h", p=128))
            nc.sync.dma_start(wk_st[:], wk_d.rearrange("(j p) h -> p j h", p=128))
            nc.sync.dma_start(wv_st[:], wv_d.rearrange("(j p) h -> p j h", p=128))
            nc.sync.dma_start(wo_st[:], wo_d[:])

            wqk_b = const.tile([128, DJ, 2 * H], BF16, tag="wqkb")
            wv_b = const.tile([128, DJ, H], BF16, tag="wvb")
            wo_b = const.tile([H, D], BF16, tag="wob")
            nc.vector.tensor_copy(wqk_b[:, :, 0:H], wq_st[:])
            nc.vector.tensor_copy(wqk_b[:, :, H : 2 * H], wk_st[:])
            nc.vector.tensor_copy(wv_b[:], wv_st[:])
            nc.vector.tensor_copy(wo_b[:], wo_st[:])

            # rotating state (fp32 accumulator + bf16 copy for matmuls)
            s_f32_prev = None
            s_bf_prev = None

            c_glob = 0
            for b in range(N_BLK):
                t0 = b * BLK
                # ---- load x block, transpose on PE, cast to bf16 ----
                xt_b = xtbf.tile([128, DJ, N_CH, 128], BF16, tag="xt")
                for ci in range(N_CH):
                    x_sb = xin.tile([128, D], F32, tag="x")
                    nc.sync.dma_start(x_sb[:], x_d[t0 + ci * C : t0 + (ci + 1) * C, :])
                    for half in range(2):
                        xt_ps = ps_xt.tile([128, 4, 128], F32, tag="xtp")
                        for jj in range(4):
                            j = half * 4 + jj
                            nc.tensor.transpose(
                                xt_ps[:, jj, :],
                                x_sb[:, j * 128 : (j + 1) * 128],
                                ident_f[:],
                            )
                        nc.scalar.copy(
                            xt_b[:, half * 4 : half * 4 + 4, ci, :], xt_ps[:]
                        )

                # ---- q/k projection: [Wq|Wk]^T x^T -> [128, BLK] psum ----
                qk_ps = ps_qk.tile([128, BLK], F32, tag="qk")
                for j in range(DJ):
                    nc.tensor.matmul(
                        qk_ps[:],
                        wqk_b[:, j, :],
                        xt_b[:, j, :, :],
                        start=(j == 0),
                        stop=(j == DJ - 1),
                    )

                # ---- phi = relu(x) + exp(min(x,0)), split into q/k tiles ----
                t1 = phip.tile([128, BLK], F32, tag="t1")
                nc.scalar.activation(t1[:], qk_ps[:], AF.Relu, scale=-1.0)
                t2 = phip.tile([128, BLK], F32, tag="t2")
                nc.scalar.activation(t2[:], t1[:], AF.Exp, scale=-1.0)
                t3 = phip.tile([128, BLK], F32, tag="t3")
                nc.vector.tensor_scalar_max(t3[:], qk_ps[:], 0.0)
                q_phi = phip.tile([H, BLK], BF16, tag="qphi")
                k_phi = phip.tile([H, BLK], BF16, tag="kphi")
                nc.vector.tensor_add(q_phi[:], t2[0:H, :], t3[0:H, :])
                nc.vector.tensor_add(k_phi[:], t2[H:128, :], t3[H:128, :])

                # ---- per-chunk recurrence ----
                # o_t = sum_{s<=t} k_s (v_s . q_t) / (z_t . q_t)  (reference's
                # einsum contracts q against v in the numerator, k in denom)
                for ci in range(N_CH):
                    cs = slice(ci * C, (ci + 1) * C)
                    first = c_glob == 0

                    # K chunk in [s, h] layout via PE transpose of k_phi
                    kt_ps = ps_sm.tile([128, H], BF16, tag="sm")
                    nc.tensor.transpose(
                        kt_ps[:], k_phi[:, cs], ident_b[0:H, 0:H]
                    )
                    k_sb = chp.tile([128, H], BF16, tag="ksb")
                    nc.scalar.copy(k_sb[:], kt_ps[:])

                    # V chunk [s, h] via 8 accumulated matmuls, + transpose
                    v_ps = ps_sm.tile([128, H], F32, tag="sm")
                    for j in range(DJ):
                        nc.tensor.matmul(
                            v_ps[:],
                            xt_b[:, j, ci, :],
                            wv_b[:, j, :],
                            start=(j == 0),
                            stop=(j == DJ - 1),
                        )
                    v_sb = chp.tile([128, H], BF16, tag="vsb")
                    nc.scalar.copy(v_sb[:], v_ps[:])
                    vt_ps = ps_sm.tile([H, 128], BF16, tag="sm")
                    nc.tensor.transpose(vt_ps[:], v_sb[:], ident_b[:])
                    vt_sb = chp.tile([H, 128], BF16, tag="vtsb")
                    nc.scalar.copy(vt_sb[:], vt_ps[:])

                    # Av^T[s,t] = v_s . q_t  (numerator); Ak^T[s,t] = k_s . q_t
                    av_ps = ps_sm.tile([128, 128], F32, tag="sm")
                    nc.tensor.matmul(
                        av_ps[:], vt_sb[:], q_phi[:, cs], start=True, stop=True
                    )
                    av_m = chp.tile([128, 128], BF16, tag="avm")
                    nc.vector.tensor_mul(av_m[:], av_ps[:], mask[:])

                    ak_ps = ps_sm.tile([128, 128], F32, tag="sm")
                    nc.tensor.matmul(
                        ak_ps[:], k_phi[:, cs], q_phi[:, cs], start=True, stop=True
                    )
                    ak_m = chp.tile([128, 128], BF16, tag="akm")
                    nc.vector.tensor_mul(ak_m[:], ak_ps[:], mask[:])

                    # O^T[i,t] = sum_s k_s[i] Av_m[s,t] + sum_j S_vk[j,i] q_t[j]
                    ot_ps = ps_sm.tile([H, 128], F32, tag="sm")
                    nc.tensor.matmul(
                        ot_ps[:], k_sb[:], av_m[:], start=True, stop=first
                    )
                    if not first:
                        nc.tensor.matmul(
                            ot_ps[:],
                            s_bf_prev[:, 0:H],
                            q_phi[:, cs],
                            start=False,
                            stop=True,
                        )
                    o_sc = chp.tile([H, 128], BF16, tag="osc")
                    nc.scalar.copy(o_sc[:], ot_ps[:])

                    # denom column: sum_s Ak_m[s,t] + z_prev . q_t
                    d_ps = ps_sm.tile([128, 1], F32, tag="sm")
                    nc.tensor.matmul(
                        d_ps[:], ak_m[:], ones_b[:], start=True, stop=first
                    )
                    if not first:
                        nc.tensor.matmul(
                            d_ps[:],
                            q_phi[:, cs],
                            s_bf_prev[:, H : H + 1],
                            start=False,
                            stop=True,
                        )
                    r_col = chp.tile([128, 1], F32, tag="rcol")
                    nc.vector.tensor_scalar_max(r_col[:], d_ps[:], 1e-6)
                    nc.vector.reciprocal(r_col[:], r_col[:])

                    # state: dS[j,i] = sum_s v_s[j] k_s[i]; dz[i] = sum_s k_s[i]
                    ds_ps = ps_sm.tile([H, H + 1], F32, tag="sm")
                    nc.tensor.matmul(
                        ds_ps[:, 0:H], v_sb[:], k_sb[:], start=True, stop=True
                    )
                    nc.tensor.matmul(
                        ds_ps[:, H : H + 1], k_sb[:], ones_b[:], start=True, stop=True
                    )
                    s_f32 = stp.tile([H, H + 1], F32, tag="sf")
                    if first:
                        nc.vector.tensor_copy(s_f32[:], ds_ps[:])
                    else:
                        nc.vector.tensor_add(s_f32[:], ds_ps[:], s_f32_prev[:])
                    s_bf = stp.tile([H, H + 1], BF16, tag="sb")
                    nc.vector.tensor_copy(s_bf[:], s_f32[:])
                    s_f32_prev, s_bf_prev = s_f32, s_bf

                    # output projection + normalization on eviction
                    for half in range(2):
                        nd = slice(half * 512, (half + 1) * 512)
                        y_ps = ps_y.tile([128, 512], F32, tag="y")
                        nc.tensor.matmul(
                            y_ps[:], o_sc[:], wo_b[:, nd], start=True, stop=True
                        )
                        y_sb = yp.tile([128, 512], F32, tag="ysb")
                        nc.vector.tensor_scalar_mul(y_sb[:], y_ps[:], r_col[:, 0:1])
                        nc.sync.dma_start(
                            y_d[t0 + ci * C : t0 + (ci + 1) * C, nd], y_sb[:]
                        )

                    c_glob += 1

    nc.compile()
    return nc


_NC = None


def _get_nc():
    global _NC
    if _NC is None:
        _NC = build_nc()
    return _NC


def kernel(x, W_q, W_k, W_v, W_o):
    nc = _get_nc()
    x = np.ascontiguousarray(x, dtype=np.float32)
    wq = np.ascontiguousarray(W_q, dtype=np.float32)
    wk = np.ascontiguousarray(W_k, dtype=np.float32)
    wv = np.ascontiguousarray(W_v, dtype=np.float32)
    wo = np.ascontiguousarray(W_o, dtype=np.float32)
    in_maps = [
        {"x": x[b], "wq": wq, "wk": wk, "wv": wv, "wo": wo} for b in range(B)
    ]
    res = run_bass_kernel_spmd(nc, in_maps, core_ids=list(range(B)))
    return np.stack([res.results[b]["y"] for b in range(B)], axis=0)


# revision 6
# speedup vs baseline: 7987.4317x; 7987.4317x over previous
"""Dense linear attention (elu+1 feature map) Trainium2 Bass kernel.

Problem: B=8, T=4096, D=1024, H=64.
  q = phi(x @ W_q), k = phi(x @ W_k), v = x @ W_v          (phi = elu+1)
  S_t = S_{t-1} + k_t v_t^T ; z_t = z_{t-1} + k_t          (S[i,j] = sum k_i v_j)
  o_t = (S_t q_t) / max(z_t . q_t, 1e-6)                    (o_i = sum_j S[i,j] q_j)
  y = o @ W_o

Note the reference einsum contracts q against the *v* index of S in the
numerator (o_t = sum_{s<=t} k_s (v_s . q_t)) while the denominator uses
z = sum k. The chunked form (C=128) per chunk:
  Av[s,t] = v_s . q_t   (masked s<=t)      -> numerator intra
  Ak[s,t] = k_s . q_t   (masked s<=t)      -> denominator intra
  O^T[i,t] = K^T Av_m + S_vk^T q^T         (S_vk[j,i] = sum v_j k_i)
  dcol[t]  = colsum(Ak_m) + q_t . z
Data-parallel over batch: one batch element per NeuronCore (8 cores).
All matmuls bf16 with fp32 PSUM accumulation; x transposed on PE in fp32.
"""

import numpy as np

import concourse.bass as bass
import concourse.mybir as mybir
import concourse.tile as tile
from concourse import bacc
from concourse.bass_utils import run_bass_kernel_spmd
from concourse.masks import make_identity, make_upper_triangular

F32 = mybir.dt.float32
BF16 = mybir.dt.bfloat16
AF = mybir.ActivationFunctionType

B, T, D, H = 8, 4096, 1024, 64
C = 128                 # chunk (recurrence step block)
BLK = 512               # projection block: 4 chunks
N_BLK = T // BLK        # 8
N_CH = BLK // C         # 4 chunks per block
DJ = D // 128           # 8 contraction sub-tiles


def build_nc(reps=1):
    nc = bacc.Bacc("TRN2", target_bir_lowering=False, debug=False)

    x_d = nc.dram_tensor("x", [T, D], F32, kind="ExternalInput")
    wq_d = nc.dram_tensor("wq", [D, H], F32, kind="ExternalInput")
    wk_d = nc.dram_tensor("wk", [D, H], F32, kind="ExternalInput")
    wv_d = nc.dram_tensor("wv", [D, H], F32, kind="ExternalInput")
    wo_d = nc.dram_tensor("wo", [H, D], F32, kind="ExternalInput")
    y_d = nc.dram_tensor("y", [T, D], F32, kind="ExternalOutput")

    with tile.TileContext(nc) as tc:
        with (
            tc.tile_pool(name="const", bufs=1) as const,
            tc.tile_pool(name="xin", bufs=3) as xin,
            tc.tile_pool(name="xtbf", bufs=2) as xtbf,
            tc.tile_pool(name="phi", bufs=2) as phip,
            tc.tile_pool(name="chunk", bufs=3) as chp,
            tc.tile_pool(name="state", bufs=2) as stp,
            tc.tile_pool(name="yout", bufs=4) as yp,
            tc.tile_pool(name="ps_xt", bufs=2, space="PSUM") as ps_xt,
            tc.tile_pool(name="ps_qk", bufs=1, space="PSUM") as ps_qk,
            tc.tile_pool(name="ps_y", bufs=2, space="PSUM") as ps_y,
            tc.tile_pool(name="ps_sm", bufs=3, space="PSUM") as ps_sm,
        ):
            # ---- constants / weights ----
            ident_f = const.tile([128, 128], F32, tag="identf")
            make_identity(nc, ident_f[:])
            ident_b = const.tile([128, 128], BF16, tag="identb")
            make_identity(nc, ident_b[:])
            # mask[s,t] = 1.0 where s <= t (upper triangular incl diagonal)
            mask = const.tile([128, 128], F32, tag="mask")
            make_upper_triangular(nc, mask[:], val=1.0, diag=True)
            ones_b = const.tile([128, 1], BF16, tag="ones")
            nc.vector.memset(ones_b[:], 1.0)

            # weight staging (fp32) and bf16 conversion
            wq_st = const.tile([128, DJ, H], F32, tag="wqst")
            wk_st = const.tile([128, DJ, H], F32, tag="wkst")
            wv_st = const.tile([128, DJ, H], F32, tag="wvst")
            wo_st = const.tile([H, D], F32, tag="wost")
            nc.sync.dma_start(wq_st[:], wq_d.rearrange("(j p) h -> p j h", p=128))
            nc.sync.dma_start(wk_st[:], wk_d.rearrange("(j p) h -> p j h", p=128))
            nc.sync.dma_start(wv_st[:], wv_d.rearrange("(j p) h -> p j h", p=128))
            nc.sync.dma_start(wo_st[:], wo_d[:])

            wqk_b = const.tile([128, DJ, 2 * H], BF16, tag="wqkb")
            wv_b = const.tile([128, DJ, H], BF16, tag="wvb")
            wo_b = const.tile([H, D], BF16, tag="wob")
            nc.vector.tensor_copy(wqk_b[:, :, 0:H], wq_st[:])
            nc.vector.tensor_copy(wqk_b[:, :, H : 2 * H], wk_st[:])
            nc.vector.tensor_copy(wv_b[:], wv_st[:])
            nc.vector.tensor_copy(wo_b[:], wo_st[:])

            def body():
                # rotating state (fp32 accumulator + bf16 copy for matmuls)
                s_f32_prev = None
                s_bf_prev = None
                c_glob = 0
                for b in range(N_BLK):
                    t0 = b * BLK
                    # ---- load x block, transpose on PE, cast to bf16 ----
                    xt_b = xtbf.tile([128, DJ, N_CH, 128], BF16, tag="xt")
                    for ci in range(N_CH):
                        x_sb = xin.tile([128, D], F32, tag="x")
                        nc.sync.dma_start(
                            x_sb[:], x_d[t0 + ci * C : t0 + (ci + 1) * C, :]
                        )
                        for half in range(2):
                            xt_ps = ps_xt.tile([128, 4, 128], F32, tag="xtp")
                            for jj in range(4):
                                j = half * 4 + jj
                                nc.tensor.transpose(
                                    xt_ps[:, jj, :],
                                    x_sb[:, j * 128 : (j + 1) * 128],
                                    ident_f[:],
                                )
                            nc.scalar.copy(
                                xt_b[:, half * 4 : half * 4 + 4, ci, :], xt_ps[:]
                            )

                    # ---- q/k projection: [Wq|Wk]^T x^T -> [128, BLK] psum ----
                    qk_ps = ps_qk.tile([128, BLK], F32, tag="qk")
                    for j in range(DJ):
                        nc.tensor.matmul(
                            qk_ps[:],
                            wqk_b[:, j, :],
                            xt_b[:, j, :, :],
                            start=(j == 0),
                            stop=(j == DJ - 1),
                        )

                    # ---- phi = relu(x) + exp(min(x,0)), split into q/k ----
                    t1 = phip.tile([128, BLK], F32, tag="t1")
                    nc.scalar.activation(t1[:], qk_ps[:], AF.Relu, scale=-1.0)
                    t2 = phip.tile([128, BLK], F32, tag="t2")
                    nc.scalar.activation(t2[:], t1[:], AF.Exp, scale=-1.0)
                    t3 = phip.tile([128, BLK], F32, tag="t3")
                    nc.vector.tensor_scalar_max(t3[:], qk_ps[:], 0.0)
                    q_phi = phip.tile([H, BLK], BF16, tag="qphi")
                    k_phi = phip.tile([H, BLK], BF16, tag="kphi")
                    nc.vector.tensor_add(q_phi[:], t2[0:H, :], t3[0:H, :])
                    nc.vector.tensor_add(k_phi[:], t2[H:128, :], t3[H:128, :])

                    # ---- per-chunk recurrence ----
                    for ci in range(N_CH):
                        cs = slice(ci * C, (ci + 1) * C)
                        first = c_glob == 0

                        # K chunk in [s, h] layout via PE transpose of k_phi
                        kt_ps = ps_sm.tile([128, H], BF16, tag="sm")
                        nc.tensor.transpose(
                            kt_ps[:], k_phi[:, cs], ident_b[0:H, 0:H]
                        )
                        k_sb = chp.tile([128, H], BF16, tag="ksb")
                        nc.scalar.copy(k_sb[:], kt_ps[:])

                        # V chunk [s, h] via 8 accumulated matmuls, + transpose
                        v_ps = ps_sm.tile([128, H], F32, tag="sm")
                        for j in range(DJ):
                            nc.tensor.matmul(
                                v_ps[:],
                                xt_b[:, j, ci, :],
                                wv_b[:, j, :],
                                start=(j == 0),
                                stop=(j == DJ - 1),
                            )
                        v_sb = chp.tile([128, H], BF16, tag="vsb")
                        nc.scalar.copy(v_sb[:], v_ps[:])
                        vt_ps = ps_sm.tile([H, 128], BF16, tag="sm")
                        nc.tensor.transpose(vt_ps[:], v_sb[:], ident_b[:])
                        vt_sb = chp.tile([H, 128], BF16, tag="vtsb")
                        nc.scalar.copy(vt_sb[:], vt_ps[:])

                        # Av[s,t] = v_s . q_t (numerator); Ak[s,t] = k_s . q_t
                        av_ps = ps_sm.tile([128, 128], F32, tag="sm")
                        nc.tensor.matmul(
                            av_ps[:], vt_sb[:], q_phi[:, cs], start=True, stop=True
                        )
                        av_m = chp.tile([128, 128], BF16, tag="avm")
                        nc.vector.tensor_mul(av_m[:], av_ps[:], mask[:])

                        ak_ps = ps_sm.tile([128, 128], F32, tag="sm")
                        nc.tensor.matmul(
                            ak_ps[:], k_phi[:, cs], q_phi[:, cs], start=True, stop=True
                        )
                        ak_m = chp.tile([128, 128], BF16, tag="akm")
                        nc.vector.tensor_mul(ak_m[:], ak_ps[:], mask[:])

                        # O^T[i,t] = sum_s k_s[i] Av_m[s,t] + sum_j S_vk[j,i] q_t[j]
                        ot_ps = ps_sm.tile([H, 128], F32, tag="sm")
                        nc.tensor.matmul(
                            ot_ps[:], k_sb[:], av_m[:], start=True, stop=first
                        )
                        if not first:
                            nc.tensor.matmul(
                                ot_ps[:],
                                s_bf_prev[:, 0:H],
                                q_phi[:, cs],
                                start=False,
                                stop=True,
                            )
                        o_sc = chp.tile([H, 128], BF16, tag="osc")
                        nc.scalar.copy(o_sc[:], ot_ps[:])

                        # denom column: sum_s Ak_m[s,t] + z_prev . q_t
                        d_ps = ps_sm.tile([128, 1], F32, tag="sm")
                        nc.tensor.matmul(
                            d_ps[:], ak_m[:], ones_b[:], start=True, stop=first
                        )
                        if not first:
                            nc.tensor.matmul(
                                d_ps[:],
                                q_phi[:, cs],
                                s_bf_prev[:, H : H + 1],
                                start=False,
                                stop=True,
                            )
                        r_col = chp.tile([128, 1], F32, tag="rcol")
                        nc.vector.tensor_scalar_max(r_col[:], d_ps[:], 1e-6)
                        nc.vector.reciprocal(r_col[:], r_col[:])

                        # state: dS[j,i] = sum_s v_s[j] k_s[i]; dz[i] = sum k_s[i]
                        ds_ps = ps_sm.tile([H, H + 1], F32, tag="sm")
                        nc.tensor.matmul(
                            ds_ps[:, 0:H], v_sb[:], k_sb[:], start=True, stop=True
                        )
                        nc.tensor.matmul(
                            ds_ps[:, H : H + 1],
                            k_sb[:],
                            ones_b[:],
                            start=True,
                            stop=True,
                        )
                        s_f32 = stp.tile([H, H + 1], F32, tag="sf")
                        if first:
                            nc.vector.tensor_copy(s_f32[:], ds_ps[:])
                        else:
                            nc.vector.tensor_add(s_f32[:], ds_ps[:], s_f32_prev[:])
                        s_bf = stp.tile([H, H + 1], BF16, tag="sb")
                        nc.vector.tensor_copy(s_bf[:], s_f32[:])
                        s_f32_prev, s_bf_prev = s_f32, s_bf

                        # output projection + normalization on eviction
                        for half in range(2):
                            nd = slice(half * 512, (half + 1) * 512)
                            y_ps = ps_y.tile([128, 512], F32, tag="y")
                            nc.tensor.matmul(
                                y_ps[:], o_sc[:], wo_b[:, nd], start=True, stop=True
                            )
                            y_sb = yp.tile([128, 512], F32, tag="ysb")
                            nc.vector.tensor_scalar_mul(
                                y_sb[:], y_ps[:], r_col[:, 0:1]
                            )
                            nc.sync.dma_start(
                                y_d[t0 + ci * C : t0 + (ci + 1) * C, nd], y_sb[:]
                            )

                        c_glob += 1

            if reps == 1:
                body()
            else:
                with tc.For_i(0, reps, 1):
                    body()

    nc.compile()
    return nc


_NC = None


def _get_nc():
    global _NC
    if _NC is None:
        _NC = build_nc()
    return _NC


def kernel(x, W_q, W_k, W_v, W_o):
    nc = _get_nc()
    x = np.ascontiguousarray(x, dtype=np.float32)
    wq = np.ascontiguousarray(W_q, dtype=np.float32)
    wk = np.ascontiguousarray(W_k, dtype=np.float32)
    wv = np.ascontiguousarray(W_v, dtype=np.float32)
    wo = np.ascontiguousarray(W_o, dtype=np.float32)
    in_maps = [
        {"x": x[b], "wq": wq, "wk": wk, "wv": wv, "wo": wo} for b in range(B)
    ]
    res = run_bass_kernel_spmd(nc, in_maps, core_ids=list(range(B)))
    return np.stack([res.results[b]["y"] for b in range(B)], axis=0)


# revision 7
# speedup vs baseline: 11903.5956x; 1.4903x over previous
"""Dense linear attention (elu+1 feature map) Trainium2 Bass kernel.

Problem: B=8, T=4096, D=1024, H=64.
  q = phi(x @ W_q), k = phi(x @ W_k), v = x @ W_v          (phi = elu+1)
  S_t = S_{t-1} + k_t v_t^T ; z_t = z_{t-1} + k_t          (S[i,j] = sum k_i v_j)
  o_t = (S_t q_t) / max(z_t . q_t, 1e-6)                    (o_i = sum_j S[i,j] q_j)
  y = o @ W_o

Note the reference einsum contracts q against the *v* index of S in the
numerator (o_t = sum_{s<=t} k_s (v_s . q_t)) while the denominator uses
z = sum k. The chunked form (C=128) per chunk:
  Av[s,t] = v_s . q_t   (masked s<=t)      -> numerator intra
  Ak[s,t] = k_s . q_t   (masked s<=t)      -> denominator intra
  O^T[i,t] = K^T Av_m + S_vk^T q^T         (S_vk[j,i] = sum v_j k_i)
  dcol[t]  = colsum(Ak_m) + q_t . z
Data-parallel over batch: one batch element per NeuronCore (8 cores).
All matmuls bf16 with fp32 PSUM accumulation; x transposed on PE in fp32.
"""

import numpy as np

import concourse.bass as bass
import concourse.mybir as mybir
import concourse.tile as tile
from concourse import bacc
from concourse.bass_utils import run_bass_kernel_spmd
from concourse.masks import make_identity, make_upper_triangular

F32 = mybir.dt.float32
BF16 = mybir.dt.bfloat16
AF = mybir.ActivationFunctionType

B, T, D, H = 8, 4096, 1024, 64
C = 128                 # chunk (recurrence step block)
BLK = 512               # projection block: 4 chunks
N_BLK = T // BLK        # 8
N_CH = BLK // C         # 4 chunks per block
DJ = D // 128           # 8 contraction sub-tiles


def build_nc(reps=1):
    nc = bacc.Bacc("TRN2", target_bir_lowering=False, debug=False)

    x_d = nc.dram_tensor("x", [T, D], F32, kind="ExternalInput")
    wq_d = nc.dram_tensor("wq", [D, H], F32, kind="ExternalInput")
    wk_d = nc.dram_tensor("wk", [D, H], F32, kind="ExternalInput")
    wv_d = nc.dram_tensor("wv", [D, H], F32, kind="ExternalInput")
    wo_d = nc.dram_tensor("wo", [H, D], F32, kind="ExternalInput")
    y_d = nc.dram_tensor("y", [T, D], F32, kind="ExternalOutput")

    with tile.TileContext(nc) as tc:
        with (
            tc.tile_pool(name="const", bufs=1) as const,
            tc.tile_pool(name="xin", bufs=3) as xin,
            tc.tile_pool(name="xtbf", bufs=2) as xtbf,
            tc.tile_pool(name="phi", bufs=2) as phip,
            tc.tile_pool(name="chunk", bufs=3) as chp,
            tc.tile_pool(name="state", bufs=2) as stp,
            tc.tile_pool(name="yout", bufs=4) as yp,
            tc.tile_pool(name="ps_xt", bufs=2, space="PSUM") as ps_xt,
            tc.tile_pool(name="ps_qk", bufs=1, space="PSUM") as ps_qk,
            tc.tile_pool(name="ps_y", bufs=2, space="PSUM") as ps_y,
            tc.tile_pool(name="ps_sm", bufs=3, space="PSUM") as ps_sm,
        ):
            # ---- constants / weights ----
            ident_f = const.tile([128, 128], F32, tag="identf")
            make_identity(nc, ident_f[:])
            ident_b = const.tile([128, 128], BF16, tag="identb")
            make_identity(nc, ident_b[:])
            # mask[s,t] = 1.0 where s <= t (upper triangular incl diagonal)
            mask = const.tile([128, 128], F32, tag="mask")
            make_upper_triangular(nc, mask[:], val=1.0, diag=True)
            ones_b = const.tile([128, 1], BF16, tag="ones")
            nc.vector.memset(ones_b[:], 1.0)

            # weight staging (fp32) and bf16 conversion
            wq_st = const.tile([128, DJ, H], F32, tag="wqst")
            wk_st = const.tile([128, DJ, H], F32, tag="wkst")
            wv_st = const.tile([128, DJ, H], F32, tag="wvst")
            wo_st = const.tile([H, D], F32, tag="wost")
            nc.sync.dma_start(wq_st[:], wq_d.rearrange("(j p) h -> p j h", p=128))
            nc.sync.dma_start(wk_st[:], wk_d.rearrange("(j p) h -> p j h", p=128))
            nc.sync.dma_start(wv_st[:], wv_d.rearrange("(j p) h -> p j h", p=128))
            nc.sync.dma_start(wo_st[:], wo_d[:])

            wqk_b = const.tile([128, DJ, 2 * H], BF16, tag="wqkb")
            wv_b = const.tile([128, DJ, H], BF16, tag="wvb")
            wo_b = const.tile([H, D], BF16, tag="wob")
            nc.vector.tensor_copy(wqk_b[:, :, 0:H], wq_st[:])
            nc.vector.tensor_copy(wqk_b[:, :, H : 2 * H], wk_st[:])
            nc.vector.tensor_copy(wv_b[:], wv_st[:])
            nc.vector.tensor_copy(wo_b[:], wo_st[:])

            def body():
                # rotating state (fp32 accumulator + bf16 copy for matmuls)
                s_f32_prev = None
                s_bf_prev = None
                c_glob = 0
                for b in range(N_BLK):
                    t0 = b * BLK
                    # ---- load x block, transpose on PE, cast to bf16 ----
                    xt_b = xtbf.tile([128, DJ, N_CH, 128], BF16, tag="xt")
                    for ci in range(N_CH):
                        x_sb = xin.tile([128, D], F32, tag="x")
                        nc.sync.dma_start(
                            x_sb[:], x_d[t0 + ci * C : t0 + (ci + 1) * C, :]
                        )
                        for half in range(2):
                            xt_ps = ps_xt.tile([128, 4, 128], F32, tag="xtp")
                            for jj in range(4):
                                j = half * 4 + jj
                                nc.tensor.transpose(
                                    xt_ps[:, jj, :],
                                    x_sb[:, j * 128 : (j + 1) * 128],
                                    ident_f[:],
                                )
                            nc.scalar.copy(
                                xt_b[:, half * 4 : half * 4 + 4, ci, :], xt_ps[:]
                            )

                    # ---- q/k projection: [Wq|Wk]^T x^T -> [128, BLK] psum ----
                    qk_ps = ps_qk.tile([128, BLK], F32, tag="qk")
                    for j in range(DJ):
                        nc.tensor.matmul(
                            qk_ps[:],
                            wqk_b[:, j, :],
                            xt_b[:, j, :, :],
                            start=(j == 0),
                            stop=(j == DJ - 1),
                        )

                    # ---- phi = relu(x) + exp(min(x,0)), split into q/k ----
                    t1 = phip.tile([128, BLK], F32, tag="t1")
                    nc.scalar.activation(t1[:], qk_ps[:], AF.Relu, scale=-1.0)
                    t2 = phip.tile([128, BLK], F32, tag="t2")
                    nc.scalar.activation(t2[:], t1[:], AF.Exp, scale=-1.0)
                    t3 = phip.tile([128, BLK], F32, tag="t3")
                    nc.vector.tensor_scalar_max(t3[:], qk_ps[:], 0.0)
                    q_phi = phip.tile([H, BLK], BF16, tag="qphi")
                    k_phi = phip.tile([H, BLK], BF16, tag="kphi")
                    nc.vector.tensor_add(q_phi[:], t2[0:H, :], t3[0:H, :])
                    nc.vector.tensor_add(k_phi[:], t2[H:128, :], t3[H:128, :])

                    # ---- per-chunk recurrence ----
                    for ci in range(N_CH):
                        cs = slice(ci * C, (ci + 1) * C)
                        first = c_glob == 0

                        # K chunk in [s, h] layout via PE transpose of k_phi
                        kt_ps = ps_sm.tile([128, H], BF16, tag="sm")
                        nc.tensor.transpose(
                            kt_ps[:], k_phi[:, cs], ident_b[0:H, 0:H]
                        )
                        k_sb = chp.tile([128, H], BF16, tag="ksb")
                        nc.scalar.copy(k_sb[:], kt_ps[:])

                        # V chunk [s, h] via 8 accumulated matmuls, + transpose
                        v_ps = ps_sm.tile([128, H], F32, tag="sm")
                        for j in range(DJ):
                            nc.tensor.matmul(
                                v_ps[:],
                                xt_b[:, j, ci, :],
                                wv_b[:, j, :],
                                start=(j == 0),
                                stop=(j == DJ - 1),
                            )
                        v_sb = chp.tile([128, H], BF16, tag="vsb")
                        nc.scalar.copy(v_sb[:], v_ps[:])
                        vt_ps = ps_sm.tile([H, 128], BF16, tag="sm")
                        nc.tensor.transpose(vt_ps[:], v_sb[:], ident_b[:])
                        vt_sb = chp.tile([H, 128], BF16, tag="vtsb")
                        nc.scalar.copy(vt_sb[:], vt_ps[:])

                        # Av[s,t] = v_s . q_t (numerator); Ak[s,t] = k_s . q_t
                        av_ps = ps_sm.tile([128, 128], F32, tag="sm")
                        nc.tensor.matmul(
                            av_ps[:], vt_sb[:], q_phi[:, cs], start=True, stop=True
                        )
                        av_m = chp.tile([128, 128], BF16, tag="avm")
                        nc.vector.tensor_mul(av_m[:], av_ps[:], mask[:])

                        ak_ps = ps_sm.tile([128, 128], F32, tag="sm")
                        nc.tensor.matmul(
                            ak_ps[:], k_phi[:, cs], q_phi[:, cs], start=True, stop=True
                        )
                        ak_m = chp.tile([128, 128], BF16, tag="akm")
                        nc.vector.tensor_mul(ak_m[:], ak_ps[:], mask[:])

                        # O^T[i,t] = sum_s k_s[i] Av_m[s,t] + sum_j S_vk[j,i] q_t[j]
                        ot_ps = ps_sm.tile([H, 128], F32, tag="sm")
                        nc.tensor.matmul(
                            ot_ps[:], k_sb[:], av_m[:], start=True, stop=first
                        )
                        if not first:
                            nc.tensor.matmul(
                                ot_ps[:],
                                s_bf_prev[:, 0:H],
                                q_phi[:, cs],
                                start=False,
                                stop=True,
                            )
                        o_sc = chp.tile([H, 128], BF16, tag="osc")
                        nc.scalar.copy(o_sc[:], ot_ps[:])

                        # denom column: sum_s Ak_m[s,t] + z_prev . q_t
                        d_ps = ps_sm.tile([128, 1], F32, tag="sm")
                        nc.tensor.matmul(
                            d_ps[:], ak_m[:], ones_b[:], start=True, stop=first
                        )
                        if not first:
                            nc.tensor.matmul(
                                d_ps[:],
                                q_phi[:, cs],
                                s_bf_prev[:, H : H + 1],
                                start=False,
                                stop=True,
                            )
                        r_col = chp.tile([128, 1], F32, tag="rcol")
                        nc.vector.tensor_scalar_max(r_col[:], d_ps[:], 1e-6)
                        nc.vector.reciprocal(r_col[:], r_col[:])

                        # state: dS[j,i] = sum_s v_s[j] k_s[i]; dz[i] = sum k_s[i]
                        ds_ps = ps_sm.tile([H, H + 1], F32, tag="sm")
                        nc.tensor.matmul(
                            ds_ps[:, 0:H], v_sb[:], k_sb[:], start=True, stop=True
                        )
                        nc.tensor.matmul(
                            ds_ps[:, H : H + 1],
                            k_sb[:],
                            ones_b[:],
                            start=True,
                            stop=True,
                        )
                        s_f32 = stp.tile([H, H + 1], F32, tag="sf")
                        if first:
                            nc.vector.tensor_copy(s_f32[:], ds_ps[:])
                        else:
                            nc.vector.tensor_add(s_f32[:], ds_ps[:], s_f32_prev[:])
                        s_bf = stp.tile([H, H + 1], BF16, tag="sb")
                        nc.vector.tensor_copy(s_bf[:], s_f32[:])
                        s_f32_prev, s_bf_prev = s_f32, s_bf

                        # output projection + normalization on eviction
                        for half in range(2):
                            nd = slice(half * 512, (half + 1) * 512)
                            y_ps = ps_y.tile([128, 512], F32, tag="y")
                            nc.tensor.matmul(
                                y_ps[:], o_sc[:], wo_b[:, nd], start=True, stop=True
                            )
                            y_sb = yp.tile([128, 512], F32, tag="ysb")
                            nc.vector.tensor_scalar_mul(
                                y_sb[:], y_ps[:], r_col[:, 0:1]
                            )
                            # SWDGE path: keeps output stores off the sync
                            # ring so they never block upcoming x loads
                            nc.gpsimd.dma_start(
                                y_d[t0 + ci * C : t0 + (ci + 1) * C, nd], y_sb[:]
                            )

                        c_glob += 1

            if reps == 1:
                body()
            else:
                with tc.For_i(0, reps, 1):
                    body()

    nc.compile()
    return nc


_NC = None


def _get_nc():
    global _NC
    if _NC is None:
        _NC = build_nc()
    return _NC


def kernel(x, W_q, W_k, W_v, W_o):
    nc = _get_nc()
    x = np.ascontiguousarray(x, dtype=np.float32)
    wq = np.ascontiguousarray(W_q, dtype=np.float32)
    wk = np.ascontiguousarray(W_k, dtype=np.float32)
    wv = np.ascontiguousarray(W_v, dtype=np.float32)
    wo = np.ascontiguousarray(W_o, dtype=np.float32)
    in_maps = [
        {"x": x[b], "wq": wq, "wk": wk, "wv": wv, "wo": wo} for b in range(B)
    ]
    res = run_bass_kernel_spmd(nc, in_maps, core_ids=list(range(B)))
    return np.stack([res.results[b]["y"] for b in range(B)], axis=0)


# revision 11
# speedup vs baseline: 12789.2444x; 1.0744x over previous
"""Dense linear attention (elu+1 feature map) Trainium2 Bass kernel.

Problem: B=8, T=4096, D=1024, H=64.
  q = phi(x @ W_q), k = phi(x @ W_k), v = x @ W_v          (phi = elu+1)
  S_t = S_{t-1} + k_t v_t^T ; z_t = z_{t-1} + k_t          (S[i,j] = sum k_i v_j)
  o_t = (S_t q_t) / max(z_t . q_t, 1e-6)                    (o_i = sum_j S[i,j] q_j)
  y = o @ W_o

Note the reference einsum contracts q against the *v* index of S in the
numerator (o_t = sum_{s<=t} k_s (v_s . q_t)) while the denominator uses
z = sum k. The chunked form (C=128) per chunk:
  Av[s,t] = v_s . q_t   (masked s<=t)      -> numerator intra
  Ak[s,t] = k_s . q_t   (masked s<=t)      -> denominator intra
  O^T[i,t] = K^T Av_m + S_vk^T q^T         (S_vk[j,i] = sum v_j k_i)
  dcol[t]  = colsum(Ak_m) + q_t . z
Data-parallel over batch: one batch element per NeuronCore (8 cores).
All matmuls bf16 with fp32 PSUM accumulation; x transposed on PE in fp32.
"""

import numpy as np

import concourse.bass as bass
import concourse.mybir as mybir
import concourse.tile as tile
from concourse import bacc
from concourse.bass_utils import run_bass_kernel_spmd
from concourse.masks import make_identity, make_upper_triangular

F32 = mybir.dt.float32
BF16 = mybir.dt.bfloat16
AF = mybir.ActivationFunctionType

B, T, D, H = 8, 4096, 1024, 64
C = 128                 # chunk (recurrence step block)
BLK = 512               # projection block: 4 chunks
N_BLK = T // BLK        # 8
N_CH = BLK // C         # 4 chunks per block
DJ = D // 128           # 8 contraction sub-tiles


def build_nc(reps=1):
    nc = bacc.Bacc("TRN2", target_bir_lowering=False, debug=False)

    x_d = nc.dram_tensor("x", [T, D], F32, kind="ExternalInput")
    wq_d = nc.dram_tensor("wq", [D, H], F32, kind="ExternalInput")
    wk_d = nc.dram_tensor("wk", [D, H], F32, kind="ExternalInput")
    wv_d = nc.dram_tensor("wv", [D, H], F32, kind="ExternalInput")
    wo_d = nc.dram_tensor("wo", [H, D], F32, kind="ExternalInput")
    y_d = nc.dram_tensor("y", [T, D], F32, kind="ExternalOutput")

    with tile.TileContext(nc) as tc:
        with (
            tc.tile_pool(name="const", bufs=1) as const,
            tc.tile_pool(name="xin", bufs=4) as xin,
            tc.tile_pool(name="xtbf", bufs=2) as xtbf,
            tc.tile_pool(name="phi", bufs=2) as phip,
            tc.tile_pool(name="chunk", bufs=6) as chp,
            tc.tile_pool(name="state", bufs=4) as stp,
            tc.tile_pool(name="yout", bufs=6) as yp,
            tc.tile_pool(name="ps_xt", bufs=1, space="PSUM") as ps_xt,
            tc.tile_pool(name="ps_qk", bufs=1, space="PSUM") as ps_qk,
            tc.tile_pool(name="ps_y", bufs=2, space="PSUM") as ps_y,
            tc.tile_pool(name="ps_sm", bufs=4, space="PSUM") as ps_sm,
        ):
            # ---- constants / weights ----
            ident_f = const.tile([128, 128], F32, tag="identf")
            make_identity(nc, ident_f[:])
            ident_b = const.tile([128, 128], BF16, tag="identb")
            make_identity(nc, ident_b[:])
            # mask[s,t] = 1.0 where s <= t (upper triangular incl diagonal)
            mask = const.tile([128, 128], F32, tag="mask")
            make_upper_triangular(nc, mask[:], val=1.0, diag=True)
            ones_b = const.tile([128, 1], BF16, tag="ones")
            nc.vector.memset(ones_b[:], 1.0)

            # weight staging (fp32) and bf16 conversion
            wq_st = const.tile([128, DJ, H], F32, tag="wqst")
            wk_st = const.tile([128, DJ, H], F32, tag="wkst")
            wv_st = const.tile([128, DJ, H], F32, tag="wvst")
            wo_st = const.tile([H, D], F32, tag="wost")
            nc.sync.dma_start(wq_st[:], wq_d.rearrange("(j p) h -> p j h", p=128))
            nc.sync.dma_start(wk_st[:], wk_d.rearrange("(j p) h -> p j h", p=128))
            nc.sync.dma_start(wv_st[:], wv_d.rearrange("(j p) h -> p j h", p=128))
            nc.sync.dma_start(wo_st[:], wo_d[:])

            wqk_b = const.tile([128, DJ, 2 * H], BF16, tag="wqkb")
            wv_b = const.tile([128, DJ, H], BF16, tag="wvb")
            wo_b = const.tile([H, D], BF16, tag="wob")
            nc.vector.tensor_copy(wqk_b[:, :, 0:H], wq_st[:])
            nc.vector.tensor_copy(wqk_b[:, :, H : 2 * H], wk_st[:])
            nc.vector.tensor_copy(wv_b[:], wv_st[:])
            nc.vector.tensor_copy(wo_b[:], wo_st[:])

            def emit_front(b):
                    t0 = b * BLK
                    # ---- load x block, transpose on PE, cast to bf16 ----
                    xt_b = xtbf.tile([128, DJ, N_CH, 128], BF16, tag="xt")
                    for ci in range(N_CH):
                        x_sb = xin.tile([128, D], F32, tag="x")
                        nc.sync.dma_start(
                            x_sb[:], x_d[t0 + ci * C : t0 + (ci + 1) * C, :]
                        )
                        for half in range(2):
                            xt_ps = ps_xt.tile([128, 4, 128], F32, tag="xtp")
                            for jj in range(4):
                                j = half * 4 + jj
                                nc.tensor.transpose(
                                    xt_ps[:, jj, :],
                                    x_sb[:, j * 128 : (j + 1) * 128],
                                    ident_f[:],
                                )
                            nc.scalar.copy(
                                xt_b[:, half * 4 : half * 4 + 4, ci, :], xt_ps[:]
                            )

                    # ---- q/k projection: [Wq|Wk]^T x^T -> [128, BLK] psum ----
                    qk_ps = ps_qk.tile([128, BLK], F32, tag="qk")
                    for j in range(DJ):
                        nc.tensor.matmul(
                            qk_ps[:],
                            wqk_b[:, j, :],
                            xt_b[:, j, :, :],
                            start=(j == 0),
                            stop=(j == DJ - 1),
                        )

                    # ---- phi = relu(x) + exp(min(x,0)), split into q/k ----
                    t1 = phip.tile([128, BLK], F32, tag="t1")
                    nc.scalar.activation(t1[:], qk_ps[:], AF.Relu, scale=-1.0)
                    t2 = phip.tile([128, BLK], F32, tag="t2")
                    nc.scalar.activation(t2[:], t1[:], AF.Exp, scale=-1.0)
                    t3 = phip.tile([128, BLK], F32, tag="t3")
                    nc.vector.tensor_scalar_max(t3[:], qk_ps[:], 0.0)
                    q_phi = phip.tile([H, BLK], BF16, tag="qphi")
                    k_phi = phip.tile([H, BLK], BF16, tag="kphi")
                    nc.vector.tensor_add(q_phi[:], t2[0:H, :], t3[0:H, :])
                    nc.vector.tensor_add(k_phi[:], t2[H:128, :], t3[H:128, :])
                    return t0, xt_b, q_phi, k_phi

            def body():
                # rotating state (fp32 accumulator + bf16 copy for matmuls)
                st = {"s_f32": None, "s_bf": None, "c_glob": 0}

                def emit_chunks(front):
                    t0, xt_b, q_phi, k_phi = front
                    s_f32_prev = st["s_f32"]
                    s_bf_prev = st["s_bf"]
                    c_glob = st["c_glob"]
                    # ---- per-chunk recurrence ----
                    for ci in range(N_CH):
                        cs = slice(ci * C, (ci + 1) * C)
                        first = c_glob == 0

                        # K chunk in [s, h] layout via PE transpose of k_phi
                        kt_ps = ps_sm.tile([128, H], BF16, tag="sm")
                        nc.tensor.transpose(
                            kt_ps[:], k_phi[:, cs], ident_b[0:H, 0:H]
                        )
                        k_sb = chp.tile([128, H], BF16, tag="ksb")
                        nc.scalar.copy(k_sb[:], kt_ps[:])

                        # V chunk [s, h] via 8 accumulated matmuls, + transpose
                        v_ps = ps_sm.tile([128, H], F32, tag="sm")
                        for j in range(DJ):
                            nc.tensor.matmul(
                                v_ps[:],
                                xt_b[:, j, ci, :],
                                wv_b[:, j, :],
                                start=(j == 0),
                                stop=(j == DJ - 1),
                            )
                        v_sb = chp.tile([128, H], BF16, tag="vsb")
                        nc.scalar.copy(v_sb[:], v_ps[:])
                        vt_ps = ps_sm.tile([H, 128], BF16, tag="sm")
                        nc.tensor.transpose(vt_ps[:], v_sb[:], ident_b[:])
                        vt_sb = chp.tile([H, 128], BF16, tag="vtsb")
                        nc.scalar.copy(vt_sb[:], vt_ps[:])

                        # Av[s,t] = v_s . q_t (numerator); Ak[s,t] = k_s . q_t
                        av_ps = ps_sm.tile([128, 128], F32, tag="sm")
                        nc.tensor.matmul(
                            av_ps[:], vt_sb[:], q_phi[:, cs], start=True, stop=True
                        )
                        av_m = chp.tile([128, 128], BF16, tag="avm")
                        nc.vector.tensor_mul(av_m[:], av_ps[:], mask[:])

                        ak_ps = ps_sm.tile([128, 128], F32, tag="sm")
                        nc.tensor.matmul(
                            ak_ps[:], k_phi[:, cs], q_phi[:, cs], start=True, stop=True
                        )
                        ak_m = chp.tile([128, 128], BF16, tag="akm")
                        nc.vector.tensor_mul(ak_m[:], ak_ps[:], mask[:])

                        # O^T[i,t] = sum_s k_s[i] Av_m[s,t] + sum_j S_vk[j,i] q_t[j]
                        ot_ps = ps_sm.tile([H, 128], F32, tag="sm")
                        nc.tensor.matmul(
                            ot_ps[:], k_sb[:], av_m[:], start=True, stop=first
                        )
                        if not first:
                            nc.tensor.matmul(
                                ot_ps[:],
                                s_bf_prev[:, 0:H],
                                q_phi[:, cs],
                                start=False,
                                stop=True,
                            )
                        o_sc = chp.tile([H, 128], BF16, tag="osc")
                        nc.scalar.copy(o_sc[:], ot_ps[:])

                        # denom column: sum_s Ak_m[s,t] + z_prev . q_t
                        d_ps = ps_sm.tile([128, 1], F32, tag="sm")
                        nc.tensor.matmul(
                            d_ps[:], ak_m[:], ones_b[:], start=True, stop=first
                        )
                        if not first:
                            nc.tensor.matmul(
                                d_ps[:],
                                q_phi[:, cs],
                                s_bf_prev[:, H : H + 1],
                                start=False,
                                stop=True,
                            )
                        r_col = chp.tile([128, 1], F32, tag="rcol")
                        nc.vector.tensor_scalar_max(r_col[:], d_ps[:], 1e-6)
                        nc.vector.reciprocal(r_col[:], r_col[:])

                        # state: dS[j,i] = sum_s v_s[j] k_s[i]; dz[i] = sum k_s[i]
                        ds_ps = ps_sm.tile([H, H + 1], F32, tag="sm")
                        nc.tensor.matmul(
                            ds_ps[:, 0:H], v_sb[:], k_sb[:], start=True, stop=True
                        )
                        nc.tensor.matmul(
                            ds_ps[:, H : H + 1],
                            k_sb[:],
                            ones_b[:],
                            start=True,
                            stop=True,
                        )
                        s_f32 = stp.tile([H, H + 1], F32, tag="sf")
                        if first:
                            nc.vector.tensor_copy(s_f32[:], ds_ps[:])
                        else:
                            nc.vector.tensor_add(s_f32[:], ds_ps[:], s_f32_prev[:])
                        s_bf = stp.tile([H, H + 1], BF16, tag="sb")
                        nc.vector.tensor_copy(s_bf[:], s_f32[:])
                        s_f32_prev, s_bf_prev = s_f32, s_bf

                        # output projection + normalization on eviction
                        for half in range(2):
                            nd = slice(half * 512, (half + 1) * 512)
                            y_ps = ps_y.tile([128, 512], F32, tag="y")
                            nc.tensor.matmul(
                                y_ps[:], o_sc[:], wo_b[:, nd], start=True, stop=True
                            )
                            y_sb = yp.tile([128, 512], F32, tag="ysb")
                            nc.vector.tensor_scalar_mul(
                                y_sb[:], y_ps[:], r_col[:, 0:1]
                            )
                            # SWDGE path: keeps output stores off the sync
                            # ring so they never block upcoming x loads
                            nc.gpsimd.dma_start(
                                y_d[t0 + ci * C : t0 + (ci + 1) * C, nd], y_sb[:]
                            )

                        c_glob += 1

                    st["s_f32"] = s_f32_prev
                    st["s_bf"] = s_bf_prev
                    st["c_glob"] = c_glob

                # software pipeline: emit block b+1's load/transpose/
                # projection/phi ahead of block b's serial chunk chain so
                # the scheduler has independent PE work to fill stalls
                front = emit_front(0)
                for b in range(1, N_BLK):
                    nxt = emit_front(b)
                    emit_chunks(front)
                    front = nxt
                emit_chunks(front)

            if reps == 1:
                body()
            else:
                with tc.For_i(0, reps, 1):
                    body()

    nc.compile()
    return nc


_NC = None


def _get_nc():
    global _NC
    if _NC is None:
        _NC = build_nc()
    return _NC


def kernel(x, W_q, W_k, W_v, W_o):
    nc = _get_nc()
    x = np.ascontiguousarray(x, dtype=np.float32)
    wq = np.ascontiguousarray(W_q, dtype=np.float32)
    wk = np.ascontiguousarray(W_k, dtype=np.float32)
    wv = np.ascontiguousarray(W_v, dtype=np.float32)
    wo = np.ascontiguousarray(W_o, dtype=np.float32)
    in_maps = [
        {"x": x[b], "wq": wq, "wk": wk, "wv": wv, "wo": wo} for b in range(B)
    ]
    res = run_bass_kernel_spmd(nc, in_maps, core_ids=list(range(B)))
    return np.stack([res.results[b]["y"] for b in range(B)], axis=0)
